# revision 18
# baseline (speedup 1.0000x reference)
"""ColorHistogramLoss Trainium2 kernel (8 NeuronCores, data-parallel).

Strategy: shard batch (32 -> 4 per core); each core streams its 25MB of
pixels through SBUF as 8 iterations of [128, 2048] plane-triples (4 real +
4 fake).  Counting is split across four engines so the DVE (the old
bottleneck) only carries what no other engine can:

- hue (9 edges): geometric ray tests on (u=g-b, v=b-r) as 4 dual-packed
  f32 custom-DVE ops (antipodal ray pairs share a boundary line; the
  sign(u) gate routes counts into a cnt + 4096*cnt packed accumulator)
  plus an E = #[u>=0] sign-sum on ScalarE.  f32 because the custom-DVE
  uop pipeline miscomputes on fp16 operands (measured).
- sat (9 edges): sat < c  <=>  mn/mx > 1-c.  ScalarE computes
  rmx = reciprocal(mx16); VectorE forms ratio16 = mn16 * rmx and then
  nine fp16 tensor_scalar indicator tiles (no accum -> 4x DVE mode,
  ~0.6us each); the TENSOR engine colsums each indicator via one-hot
  stationaries into a PSUM accumulator (216ns per 512-chunk, running
  concurrently with the DVE at no cost), accumulated over all 8
  iterations; one DVE reduce at the end reads the 18 totals.
- val (9 edges): Sign activations on fp16 mx pairs on ScalarE with fused
  accumulation (host decodes N_lt = (N - sign_sum)/2), batched over
  iteration pairs to amortize the fixed activation cost.
- min/max chains run in fp16 on the DVE at 2x (r16/g16/b16 casts ride
  the ScalarE Copy activation; Sign/Reciprocal/Copy share one act table
  so there is a single table load).

All counts are exact integers; only boundary-ulp pixels (fp16 rounding
of mx/mn/ratio) differ from the f32 reference (rel err ~6e-3 measured
against the reference loss, gate is 2e-2).
"""

import sys

if "/opt/trn_rl_repo" not in sys.path:
    sys.path.insert(0, "/opt/trn_rl_repo")

import numpy as np

from concourse import bacc, mybir, tile
from concourse import bass_utils

# ---- problem constants (hardcoded; kernel.py must be self-contained) ----
B, C, H, W = 32, 3, 512, 512
NCORES = 8
BPC = B // NCORES            # batches per core
P, F = 128, 2048             # SBUF tile: one [512,512] plane = [128, 2048]
NITER = 2 * BPC              # 4 real + 4 fake plane-triple iterations
ACCW = 20                    # accumulator columns per iteration
NPIX = B * H * W             # pixels per full histogram
ALPHA, BETA, GAMMA = 0.3, 0.4, 0.4

AF = mybir.AluOpType
F32 = mybir.dt.float32
F16 = mybir.dt.float16

LAST_EXEC_NS = None
_CACHE = {}

PACK = 4096.0  # dual-count packing: accum = cntA + PACK*cntB (exact in f32)
C23 = float(np.float32(2.0) / np.float32(3.0))

SVAL_EDGES = (0.1, 0.2, 0.3, 0.4, 0.5, 0.6, 0.7, 0.8, 0.9)
SAT_EDGES = (0.1, 0.2, 0.3, 0.4, 0.5, 0.6, 0.7, 0.8, 0.9)
NSAT = len(SAT_EDGES)
NK = NSAT + 1                 # PE psum rows: 9 sat edges + val-0.5


def _register_custom_ops():
    """Author + register fused DVE ops in the dve_ops registry at runtime
    (the repo list is read-only; registration is by-name so appending to the
    module-level OPS list is sufficient for table-gen and tracing)."""
    from concourse import dve_ops
    from concourse.dve_spec import (
        C0, C1, C2, Spec, Src0, Src1, Zero, One, _has_src1, lower, maxx,
        minn, select,
    )
    from concourse.dve_uop import DveOpSpec

    if hasattr(dve_ops, "HPA"):
        return dve_ops

    from operator import add as _add

    def _accref(body_fn):
        def ref(in0, in1, c0, c1, c2):
            b = body_fn(
                np.asarray(in0, np.float32),
                np.asarray(in1, np.float32) if in1 is not None else None,
                c0, c1, c2,
            ).astype(np.float32)
            return b, b.reshape(b.shape[0], -1).sum(axis=-1, keepdims=True)
        return ref

    # gate: 1 where u>=0 else PACK (routes the count into the high field)
    gate = select(Src0 >= Zero, One, C1)

    defs = [
        # hue pair, A-form: t = (Src0 + C0*Src1 <= 0); accum t*(1|C1 by sign)
        (
            "HPA",
            Spec(
                body=((Src0 + C0 * Src1) <= Zero) * gate,
                accum=_add,
                accum_init=Zero,
                reference=_accref(
                    lambda u, v, c0, c1, c2: ((u + np.float32(c0) * v) <= 0)
                    * np.where(u >= 0, 1.0, c1)
                ),
            ),
        ),
        # hue pair, B-form: t = (Src1 + C0*Src0 <= 0)
        (
            "HPB",
            Spec(
                body=((Src1 + C0 * Src0) <= Zero) * gate,
                accum=_add,
                accum_init=Zero,
                reference=_accref(
                    lambda u, v, c0, c1, c2: ((v + np.float32(c0) * u) <= 0)
                    * np.where(u >= 0, 1.0, c1)
                ),
            ),
        ),
    ]
    for name, spec in defs:
        row = 1 + len(dve_ops.OPS)
        shas = {}
        for ver in ("v3", "v4"):
            uops = lower(spec, ver=ver)
            shas[ver] = DveOpSpec(
                name=name, opcode=row, uops=uops, rd1_en=_has_src1(spec)
            ).sha(ver)
        op = dve_ops.DveOp(name, spec, False, uops_sha=shas)
        dve_ops.OPS.append(op)
        dve_ops.CUSTOM_DVE_SPECS[name] = spec
        dve_ops._SUB_OPCODE_FOR_NAME[name] = row
        setattr(dve_ops, name, op)
    return dve_ops


def _build():
    dve_ops = _register_custom_ops()
    nc = bacc.Bacc(
        "TRN2", target_bir_lowering=False, debug=False, num_devices=NCORES
    )
    xr = nc.dram_tensor("x_real", [BPC * C * P, F], F32, kind="ExternalInput").ap()
    xf = nc.dram_tensor("x_fake", [BPC * C * P, F], F32, kind="ExternalInput").ap()
    out = nc.dram_tensor("out", [NITER * P, ACCW], F32, kind="ExternalOutput").ap()
    sat_out = nc.dram_tensor("sat", [P, 2], F32, kind="ExternalOutput").ap()

    SIGN = mybir.ActivationFunctionType.Sign
    COPY = mybir.ActivationFunctionType.Copy
    LN = mybir.ActivationFunctionType.Ln

    with tile.TileContext(nc) as tc:
        with tc.tile_pool(name="main", bufs=2) as io_pool, tc.tile_pool(
            name="tmp", bufs=1
        ) as tmp_pool, tc.tile_pool(name="ps", bufs=1, space="PSUM") as ppool:
            # per-edge bias tiles for ScalarE Sign activations (bias = -edge)
            sbias = []
            for e in SVAL_EDGES + (0.0,):       # 0.0: E-count sign(u)
                bt = tmp_pool.tile([P, 1], F32, tag=f"sb{int(e*10)}",
                                   name=f"sb{int(e*10)}")
                nc.gpsimd.memset(bt[:], -e)
                sbias.append(bt)
            # one-hot stationaries for the PE colsum (col j = ones)
            stats = []
            for j in range(NK):
                st = tmp_pool.tile([P, NK], F16, tag=f"st{j}", name=f"st{j}")
                nc.vector.memset(st[:], 0.0)
                nc.vector.memset(st[:, j : j + 1], 1.0)
                stats.append(st)
            psum = ppool.tile([P, F], F32, tag="psum")
            scr2b = tmp_pool.tile([P, 2, F], F16, tag="scr2b", name="scr2b")
            ind_rot = [
                tmp_pool.tile([P, F], F16, tag=f"ind{k}", name=f"ind{k}")
                for k in range(4)
            ]
            acc_sat = tmp_pool.tile([P, 2], F32, tag="acc_sat", name="acc_sat")
            nc.vector.memset(acc_sat[:], 0.0)
            # f32 scratch shared by the custom-DVE dummy outs and the final
            # PSUM readback
            scr = tmp_pool.tile([P, F], F32, tag="scr", name="scr")

            V = nc.vector
            S = nc.scalar
            for it in range(NITER):
                src = xr if it < BPC else xf
                bi = it % BPC

                def plane(c):
                    q = bi * C + c
                    return src[q * P : (q + 1) * P, :]

                r = io_pool.tile([P, F], F32, tag="r")
                g = io_pool.tile([P, F], F32, tag="g")
                bl = io_pool.tile([P, F], F32, tag="bl")
                nc.sync.dma_start(g[:], plane(1))
                nc.sync.dma_start(bl[:], plane(2))
                nc.sync.dma_start(r[:], plane(0))

                # u and mx16 live in [P, 2, F] pair tiles so ScalarE can run
                # one batched activation over two iterations' data
                if it % 2 == 0:
                    upair = io_pool.tile([P, 2, F], F32, tag="upair")
                    mxpair = io_pool.tile([P, 2, F], F16, tag="mxpair")
                u = upair[:, it % 2, :]
                mx16 = mxpair[:, it % 2, :]
                v = tmp_pool.tile([P, F], F32, tag="v", name="v")
                r16 = io_pool.tile([P, F], F16, tag="r16")
                g16 = io_pool.tile([P, F], F16, tag="g16")
                b16 = io_pool.tile([P, F], F16, tag="b16")
                m1 = tmp_pool.tile([P, F], F16, tag="m1", name="m1")
                mn1 = tmp_pool.tile([P, F], F16, tag="mn1", name="mn1")
                mn16 = io_pool.tile([P, F], F16, tag="mn16")
                rmx = io_pool.tile([P, F], F16, tag="rmx")
                ratio = io_pool.tile([P, F], F16, tag="ratio")
                acc = io_pool.tile([P, ACCW], F32, tag="acc")

                # ScalarE: fp16 casts of the three planes
                S.activation(g16[:], g[:], COPY)
                S.activation(b16[:], bl[:], COPY)
                S.activation(r16[:], r[:], COPY)

                # DVE: u, v in f32 (custom-DVE hue ops need f32 operands)
                V.tensor_tensor(u, g[:], bl[:], AF.subtract)
                V.tensor_tensor(v[:], bl[:], r[:], AF.subtract)
                # hue pairs on (u, v): acc0..3
                V._custom_dve(dve_ops.HPA, out=scr[:], in0=u, in1=v[:],
                              s0=0.6, s1=PACK, accum_out=acc[:, 0:1])
                V._custom_dve(dve_ops.HPB, out=scr[:], in0=u, in1=v[:],
                              s0=0.8, s1=PACK, accum_out=acc[:, 1:2])
                V._custom_dve(dve_ops.HPB, out=scr[:], in0=u, in1=v[:],
                              s0=0.2, s1=PACK, accum_out=acc[:, 2:3])
                V._custom_dve(dve_ops.HPB, out=scr[:], in0=u, in1=v[:],
                              s0=-C23, s1=PACK, accum_out=acc[:, 3:4])
                # fp16 min/max chains (2x DVE mode)
                V.tensor_tensor(m1[:], r16[:], g16[:], AF.max)
                V.tensor_tensor(mx16, m1[:], b16[:], AF.max)
                V.tensor_tensor(mn1[:], r16[:], g16[:], AF.min)
                V.tensor_tensor(mn16[:], mn1[:], b16[:], AF.min)
                # sat < c  <=>  mn/mx > 1-c  <=>  ln(mn) - ln(mx) > ln(1-c).
                # (ScalarE Reciprocal is blocked for accuracy; Ln shares the
                # natural_log act table with Sign and Copy -> no reloads.
                # ln(0) = -inf keeps the mn==0 pixels in the right bin.)
                S.activation(rmx[:], mx16, LN)
                S.activation(ratio[:], mn16[:], LN)
                w = tmp_pool.tile([P, F], F16, tag="w", name="w")
                V.tensor_tensor(w[:], ratio[:], rmx[:], AF.subtract)
                # sat indicators (4x DVE mode, no accum) + PE colsum into PSUM
                base = 0 if it < BPC else 32
                rows = slice(base, base + NK)
                for j, c in enumerate(SAT_EDGES):
                    indt = ind_rot[j % 4][:]
                    V.tensor_scalar(indt, w[:], float(np.log(1.0 - c)), None,
                                    AF.is_gt)
                    for cj in range(4):
                        nc.tensor.matmul(
                            psum[rows, cj * 512 : (cj + 1) * 512],
                            stats[j][:, :],
                            indt[:, cj * 512 : (cj + 1) * 512],
                            start=(it % BPC == 0 and j == 0),
                            stop=False,
                        )

                # val-0.5 rides the PE path too (rebalances ScalarE -> PE)
                indt = ind_rot[NSAT % 4][:]
                V.tensor_scalar(indt, mx16, 0.5, None, AF.is_lt)
                for cj in range(4):
                    nc.tensor.matmul(
                        psum[rows, cj * 512 : (cj + 1) * 512],
                        stats[NSAT][:, :],
                        indt[:, cj * 512 : (cj + 1) * 512],
                        start=False,
                        stop=(it % BPC == BPC - 1),
                    )

                if it % 2 == 1:
                    # ScalarE, batched over the iteration pair: val-9 + E
                    # acc10..18 (val sign-sums over 2*F on fp16 mx), acc4 (E)
                    for k in range(9):
                        if k == 4:      # 0.5 is counted on the PE path
                            continue
                        S.activation(
                            scr2b[:], mxpair[:], SIGN, bias=sbias[k][:],
                            accum_out=acc[:, 10 + k : 11 + k],
                        )
                    S.activation(
                        scr2b[:], upair[:], SIGN,
                        bias=sbias[9][:], accum_out=acc[:, 4:5],
                    )
                nc.sync.dma_start(out[it * P : (it + 1) * P, :], acc[:, :])
                if it == BPC - 1:
                    # real-half PSUM rows are final; read them back now so
                    # the end-of-kernel tail only carries the fake half
                    V.tensor_scalar(scr[0:NK, :], psum[0:NK, :], 1.0, None,
                                    AF.mult, AF.add,
                                    accum_out=acc_sat[0:NK, 0:1])

            # final: read the fake-half PE accumulators out of PSUM
            V.tensor_scalar(scr[0:NK, :], psum[32 : 32 + NK, :], 1.0,
                            None, AF.mult, AF.add,
                            accum_out=acc_sat[32 : 32 + NK, 1:2])
            nc.sync.dma_start(sat_out, acc_sat[:])

    nc.compile()
    return nc


def _register_ntff_hook():
    """Register the axon NTFF profiling hook (the container's antenv stub
    lacks axon_hooks, so trn_boot's registration was skipped). Also keep
    profile artifacts local instead of uploading to a share."""
    import types

    import antenv

    if "antenv.axon_hooks" not in sys.modules:
        mod = types.ModuleType("antenv.axon_hooks")
        holder = [None]
        mod.set_axon_ntff_profile_hook = lambda h: holder.__setitem__(0, h)
        mod.get_axon_ntff_profile_hook = lambda: holder[0]
        sys.modules["antenv.axon_hooks"] = mod
        antenv.axon_hooks = mod
    from antenv import axon_hooks

    if axon_hooks.get_axon_ntff_profile_hook() is None:
        from trn_agent_boot.trn_boot import _ntff_profile_via_ctypes

        axon_hooks.set_axon_ntff_profile_hook(
            _ntff_profile_via_ctypes("/opt/axon/libaxon_pjrt.so")
        )
    bass_utils.upload_artifacts = lambda tmpdir: tmpdir


def _get_nc():
    if "nc" not in _CACHE:
        _CACHE["nc"] = _build()
    return _CACHE["nc"]


def kernel(x_real: np.ndarray, x_fake: np.ndarray) -> np.ndarray:
    global LAST_EXEC_NS
    nc = _get_nc()

    in_maps = []
    for c in range(NCORES):
        sl = slice(c * BPC, (c + 1) * BPC)
        in_maps.append(
            {
                "x_real": np.ascontiguousarray(x_real[sl]).reshape(BPC * C * P, F),
                "x_fake": np.ascontiguousarray(x_fake[sl]).reshape(BPC * C * P, F),
            }
        )

    import os

    trace = bool(int(os.environ.get("KERNEL_TRACE", "0")))
    if trace:
        _register_ntff_hook()
    res = bass_utils.run_bass_kernel_spmd(
        nc, in_maps, core_ids=list(range(NCORES)), trace=trace
    )
    LAST_EXEC_NS = res.exec_time_ns
    _CACHE["last_res"] = res

    # Decode.  Packed hue cols are exact ints in f32: split via % and //.
    # Sign-sum cols decode as N_lt = (N - S)/2.  Sat counts come from the
    # PE PSUM accumulators (already plain cumulative counts).
    PK = int(PACK)
    hue_lo = np.zeros((2, 4))      # A,B,C,D
    hue_hi = np.zeros((2, 4))      # F,G,H,I tilde counts (u<0 side)
    E_sign = np.zeros(2)
    sign_sums = np.zeros((2, 9))
    C_sat = np.zeros((2, 9))
    C_val05 = np.zeros(2)
    for core_out in res.results:
        o = np.asarray(core_out["out"]).reshape(NITER, P, ACCW).astype(np.int64)
        s = np.asarray(core_out["sat"]).astype(np.int64)
        C_sat[0] += s[0:NSAT, 0]
        C_sat[1] += s[32 : 32 + NSAT, 1]
        C_val05[0] += s[NSAT, 0]
        C_val05[1] += s[32 + NSAT, 1]
        for t, sl in ((0, slice(0, BPC)), (1, slice(BPC, NITER))):
            blk = o[sl]
            packed = blk[:, :, 0:4]
            hue_lo[t] += (packed % PK).sum(axis=(0, 1))
            hue_hi[t] += (packed // PK).sum(axis=(0, 1))
            # sign-sums live in the odd iterations of each half
            sign_sums[t] += blk[(1, 3), :, 10:19].sum(axis=(0, 1))
            E_sign[t] += blk[(1, 3), :, 4].sum() + blk[(1, 3), :, 5].sum()

    # hue cumulative counts [2, 9]
    E = (NPIX + E_sign) / 2.0
    C_hue = np.stack([
        hue_lo[:, 0], hue_lo[:, 1], hue_lo[:, 2], hue_lo[:, 3], E,
        NPIX - hue_hi[:, 0], NPIX - hue_hi[:, 1],
        NPIX - hue_hi[:, 2], NPIX - hue_hi[:, 3],
    ], axis=1)
    C_val = (NPIX - sign_sums) / 2.0
    C_val[:, 4] = C_val05

    loss = 0.0
    for wgt, Cc in ((ALPHA, C_hue), (BETA, C_sat), (GAMMA, C_val)):
        hist = np.zeros((2, 10))
        hist[:, 0] = Cc[:, 0]
        hist[:, 1:9] = Cc[:, 1:] - Cc[:, :-1]
        hist[:, 9] = NPIX - Cc[:, 8]
        loss += wgt * np.abs(hist[0] - hist[1]).mean()
    return np.asarray(loss, dtype=np.float32)


# revision 19
# speedup vs baseline: 1.0350x; 1.0350x over previous
"""ColorHistogramLoss Trainium2 kernel (8 NeuronCores, data-parallel).

Strategy: shard batch (32 -> 4 per core); each core streams its 25MB of
pixels through SBUF as 8 iterations of [128, 2048] plane-triples (4 real +
4 fake).  Counting is split across four engines so the DVE (the old
bottleneck) only carries what no other engine can:

- hue (9 edges): geometric ray tests on (u=g-b, v=b-r) as 4 dual-packed
  f32 custom-DVE ops (antipodal ray pairs share a boundary line; the
  sign(u) gate routes counts into a cnt + 4096*cnt packed accumulator)
  plus an E = #[u>=0] sign-sum on ScalarE.  f32 because the custom-DVE
  uop pipeline miscomputes on fp16 operands (measured).
- sat (9 edges): sat < c  <=>  mn/mx > 1-c.  ScalarE computes
  rmx = reciprocal(mx16); VectorE forms ratio16 = mn16 * rmx and then
  nine fp16 tensor_scalar indicator tiles (no accum -> 4x DVE mode,
  ~0.6us each); the TENSOR engine colsums each indicator via one-hot
  stationaries into a PSUM accumulator (216ns per 512-chunk, running
  concurrently with the DVE at no cost), accumulated over all 8
  iterations; one DVE reduce at the end reads the 18 totals.
- val (9 edges): Sign activations on fp16 mx pairs on ScalarE with fused
  accumulation (host decodes N_lt = (N - sign_sum)/2), batched over
  iteration pairs to amortize the fixed activation cost.
- min/max chains run in fp16 on the DVE at 2x (r16/g16/b16 casts ride
  the ScalarE Copy activation; Sign/Reciprocal/Copy share one act table
  so there is a single table load).

All counts are exact integers; only boundary-ulp pixels (fp16 rounding
of mx/mn/ratio) differ from the f32 reference (rel err ~6e-3 measured
against the reference loss, gate is 2e-2).
"""

import sys

if "/opt/trn_rl_repo" not in sys.path:
    sys.path.insert(0, "/opt/trn_rl_repo")

import numpy as np

from concourse import bacc, mybir, tile
from concourse import bass_utils

# ---- problem constants (hardcoded; kernel.py must be self-contained) ----
B, C, H, W = 32, 3, 512, 512
NCORES = 8
BPC = B // NCORES            # batches per core
P, F = 128, 2048             # SBUF tile: one [512,512] plane = [128, 2048]
NITER = 2 * BPC              # 4 real + 4 fake plane-triple iterations
ACCW = 20                    # accumulator columns per iteration
NPIX = B * H * W             # pixels per full histogram
ALPHA, BETA, GAMMA = 0.3, 0.4, 0.4

AF = mybir.AluOpType
F32 = mybir.dt.float32
F16 = mybir.dt.float16

LAST_EXEC_NS = None
_CACHE = {}

PACK = 4096.0  # dual-count packing: accum = cntA + PACK*cntB (exact in f32)
C23 = float(np.float32(2.0) / np.float32(3.0))

SVAL_EDGES = (0.1, 0.2, 0.3, 0.4, 0.5, 0.6, 0.7, 0.8, 0.9)
SAT_EDGES = (0.1, 0.2, 0.3, 0.4, 0.5, 0.6, 0.7, 0.8, 0.9)
NSAT = len(SAT_EDGES)
NK = NSAT + 1                 # PE psum rows: 9 sat edges + val-0.5


def _register_custom_ops():
    """Author + register fused DVE ops in the dve_ops registry at runtime
    (the repo list is read-only; registration is by-name so appending to the
    module-level OPS list is sufficient for table-gen and tracing)."""
    from concourse import dve_ops
    from concourse.dve_spec import (
        C0, C1, C2, Spec, Src0, Src1, Zero, One, _has_src1, lower, maxx,
        minn, select,
    )
    from concourse.dve_uop import DveOpSpec

    if hasattr(dve_ops, "HPA"):
        return dve_ops

    from operator import add as _add

    def _accref(body_fn):
        def ref(in0, in1, c0, c1, c2):
            b = body_fn(
                np.asarray(in0, np.float32),
                np.asarray(in1, np.float32) if in1 is not None else None,
                c0, c1, c2,
            ).astype(np.float32)
            return b, b.reshape(b.shape[0], -1).sum(axis=-1, keepdims=True)
        return ref

    # gate: 1 where u>=0 else PACK (routes the count into the high field)
    gate = select(Src0 >= Zero, One, C1)

    defs = [
        # hue pair, A-form: t = (Src0 + C0*Src1 <= 0); accum t*(1|C1 by sign)
        (
            "HPA",
            Spec(
                body=((Src0 + C0 * Src1) <= Zero) * gate,
                accum=_add,
                accum_init=Zero,
                reference=_accref(
                    lambda u, v, c0, c1, c2: ((u + np.float32(c0) * v) <= 0)
                    * np.where(u >= 0, 1.0, c1)
                ),
            ),
        ),
        # hue pair, B-form: t = (Src1 + C0*Src0 <= 0)
        (
            "HPB",
            Spec(
                body=((Src1 + C0 * Src0) <= Zero) * gate,
                accum=_add,
                accum_init=Zero,
                reference=_accref(
                    lambda u, v, c0, c1, c2: ((v + np.float32(c0) * u) <= 0)
                    * np.where(u >= 0, 1.0, c1)
                ),
            ),
        ),
    ]
    for name, spec in defs:
        row = 1 + len(dve_ops.OPS)
        shas = {}
        for ver in ("v3", "v4"):
            uops = lower(spec, ver=ver)
            shas[ver] = DveOpSpec(
                name=name, opcode=row, uops=uops, rd1_en=_has_src1(spec)
            ).sha(ver)
        op = dve_ops.DveOp(name, spec, False, uops_sha=shas)
        dve_ops.OPS.append(op)
        dve_ops.CUSTOM_DVE_SPECS[name] = spec
        dve_ops._SUB_OPCODE_FOR_NAME[name] = row
        setattr(dve_ops, name, op)
    return dve_ops


def _build():
    dve_ops = _register_custom_ops()
    nc = bacc.Bacc(
        "TRN2", target_bir_lowering=False, debug=False, num_devices=NCORES
    )
    xr = nc.dram_tensor("x_real", [BPC * C * P, F], F32, kind="ExternalInput").ap()
    xf = nc.dram_tensor("x_fake", [BPC * C * P, F], F32, kind="ExternalInput").ap()
    out = nc.dram_tensor("out", [NITER * P, ACCW], F32, kind="ExternalOutput").ap()
    sat_out = nc.dram_tensor("sat", [P, 2], F32, kind="ExternalOutput").ap()

    SIGN = mybir.ActivationFunctionType.Sign
    COPY = mybir.ActivationFunctionType.Copy
    LN = mybir.ActivationFunctionType.Ln

    with tile.TileContext(nc) as tc:
        with tc.tile_pool(name="main", bufs=2) as io_pool, tc.tile_pool(
            name="tmp", bufs=1
        ) as tmp_pool, tc.tile_pool(name="ps", bufs=1, space="PSUM") as ppool:
            # per-edge bias tiles for ScalarE Sign activations (bias = -edge)
            sbias = []
            for e in SVAL_EDGES + (0.0,):       # 0.0: E-count sign(u)
                bt = tmp_pool.tile([P, 1], F32, tag=f"sb{int(e*10)}",
                                   name=f"sb{int(e*10)}")
                nc.gpsimd.memset(bt[:], -e)
                sbias.append(bt)
            # one-hot stationaries for the PE colsum (col j = ones)
            stats = []
            for j in range(NK):
                st = tmp_pool.tile([P, NK], F16, tag=f"st{j}", name=f"st{j}")
                nc.vector.memset(st[:], 0.0)
                nc.vector.memset(st[:, j : j + 1], 1.0)
                stats.append(st)
            psum = ppool.tile([P, F], F32, tag="psum")
            scr2b = tmp_pool.tile([P, 2, F], F16, tag="scr2b", name="scr2b")
            ind_rot = [
                tmp_pool.tile([P, F], F16, tag=f"ind{k}", name=f"ind{k}")
                for k in range(4)
            ]
            acc_sat = tmp_pool.tile([P, 2], F32, tag="acc_sat", name="acc_sat")
            nc.vector.memset(acc_sat[:], 0.0)
            # f32 scratch shared by the custom-DVE dummy outs and the final
            # PSUM readback
            scr = tmp_pool.tile([P, F], F32, tag="scr", name="scr")

            V = nc.vector
            S = nc.scalar
            for it in range(NITER):
                src = xr if it < BPC else xf
                bi = it % BPC

                def plane(c):
                    q = bi * C + c
                    return src[q * P : (q + 1) * P, :]

                r = io_pool.tile([P, F], F32, tag="r")
                g = io_pool.tile([P, F], F32, tag="g")
                bl = io_pool.tile([P, F], F32, tag="bl")
                nc.sync.dma_start(g[:], plane(1))
                nc.sync.dma_start(bl[:], plane(2))
                nc.sync.dma_start(r[:], plane(0))

                # u and mx16 live in [P, 2, F] pair tiles so ScalarE can run
                # one batched activation over two iterations' data
                if it % 2 == 0:
                    upair = io_pool.tile([P, 2, F], F32, tag="upair")
                    mxpair = io_pool.tile([P, 2, F], F16, tag="mxpair")
                u = upair[:, it % 2, :]
                mx16 = mxpair[:, it % 2, :]
                v = tmp_pool.tile([P, F], F32, tag="v", name="v")
                r16 = io_pool.tile([P, F], F16, tag="r16")
                g16 = io_pool.tile([P, F], F16, tag="g16")
                b16 = io_pool.tile([P, F], F16, tag="b16")
                m1 = tmp_pool.tile([P, F], F16, tag="m1", name="m1")
                mn1 = tmp_pool.tile([P, F], F16, tag="mn1", name="mn1")
                mn16 = io_pool.tile([P, F], F16, tag="mn16")
                rmx = io_pool.tile([P, F], F16, tag="rmx")
                ratio = io_pool.tile([P, F], F16, tag="ratio")
                acc = io_pool.tile([P, ACCW], F32, tag="acc")

                # ScalarE: fp16 casts of the three planes
                S.activation(g16[:], g[:], COPY)
                S.activation(b16[:], bl[:], COPY)
                S.activation(r16[:], r[:], COPY)

                # DVE: u, v in f32 (custom-DVE hue ops need f32 operands)
                V.tensor_tensor(u, g[:], bl[:], AF.subtract)
                V.tensor_tensor(v[:], bl[:], r[:], AF.subtract)
                # fp16 min/max chains first (2x DVE mode): ScalarE's Ln and
                # Sign work hangs off mx16/mn16, so feed it before the long
                # hue-custom block
                V.tensor_tensor(m1[:], r16[:], g16[:], AF.max)
                V.tensor_tensor(mx16, m1[:], b16[:], AF.max)
                V.tensor_tensor(mn1[:], r16[:], g16[:], AF.min)
                V.tensor_tensor(mn16[:], mn1[:], b16[:], AF.min)
                # hue pairs on (u, v): acc0..3
                V._custom_dve(dve_ops.HPA, out=scr[:], in0=u, in1=v[:],
                              s0=0.6, s1=PACK, accum_out=acc[:, 0:1])
                V._custom_dve(dve_ops.HPB, out=scr[:], in0=u, in1=v[:],
                              s0=0.8, s1=PACK, accum_out=acc[:, 1:2])
                V._custom_dve(dve_ops.HPB, out=scr[:], in0=u, in1=v[:],
                              s0=0.2, s1=PACK, accum_out=acc[:, 2:3])
                V._custom_dve(dve_ops.HPB, out=scr[:], in0=u, in1=v[:],
                              s0=-C23, s1=PACK, accum_out=acc[:, 3:4])
                # sat < c  <=>  mn/mx > 1-c  <=>  ln(mn) - ln(mx) > ln(1-c).
                # (ScalarE Reciprocal is blocked for accuracy; Ln shares the
                # natural_log act table with Sign and Copy -> no reloads.
                # ln(0) = -inf keeps the mn==0 pixels in the right bin.)
                S.activation(rmx[:], mx16, LN)
                S.activation(ratio[:], mn16[:], LN)
                w = tmp_pool.tile([P, F], F16, tag="w", name="w")
                V.tensor_tensor(w[:], ratio[:], rmx[:], AF.subtract)
                # sat indicators (4x DVE mode, no accum) + PE colsum into PSUM
                base = 0 if it < BPC else 32
                rows = slice(base, base + NK)
                for j, c in enumerate(SAT_EDGES):
                    indt = ind_rot[j % 4][:]
                    V.tensor_scalar(indt, w[:], float(np.log(1.0 - c)), None,
                                    AF.is_gt)
                    for cj in range(4):
                        nc.tensor.matmul(
                            psum[rows, cj * 512 : (cj + 1) * 512],
                            stats[j][:, :],
                            indt[:, cj * 512 : (cj + 1) * 512],
                            start=(it % BPC == 0 and j == 0),
                            stop=False,
                        )

                # val-0.5 rides the PE path too (rebalances ScalarE -> PE)
                indt = ind_rot[NSAT % 4][:]
                V.tensor_scalar(indt, mx16, 0.5, None, AF.is_lt)
                for cj in range(4):
                    nc.tensor.matmul(
                        psum[rows, cj * 512 : (cj + 1) * 512],
                        stats[NSAT][:, :],
                        indt[:, cj * 512 : (cj + 1) * 512],
                        start=False,
                        stop=(it % BPC == BPC - 1),
                    )

                if it % 2 == 1:
                    # ScalarE, batched over the iteration pair: val-9 + E
                    # acc10..18 (val sign-sums over 2*F on fp16 mx), acc4 (E)
                    for k in range(9):
                        if k == 4:      # 0.5 is counted on the PE path
                            continue
                        S.activation(
                            scr2b[:], mxpair[:], SIGN, bias=sbias[k][:],
                            accum_out=acc[:, 10 + k : 11 + k],
                        )
                    S.activation(
                        scr2b[:], upair[:], SIGN,
                        bias=sbias[9][:], accum_out=acc[:, 4:5],
                    )
                nc.sync.dma_start(out[it * P : (it + 1) * P, :], acc[:, :])
                if it == BPC - 1:
                    # real-half PSUM rows are final; read them back now so
                    # the end-of-kernel tail only carries the fake half
                    V.tensor_scalar(scr[0:NK, :], psum[0:NK, :], 1.0, None,
                                    AF.mult, AF.add,
                                    accum_out=acc_sat[0:NK, 0:1])

            # final: read the fake-half PE accumulators out of PSUM
            V.tensor_scalar(scr[0:NK, :], psum[32 : 32 + NK, :], 1.0,
                            None, AF.mult, AF.add,
                            accum_out=acc_sat[32 : 32 + NK, 1:2])
            nc.sync.dma_start(sat_out, acc_sat[:])

    nc.compile()
    return nc


def _register_ntff_hook():
    """Register the axon NTFF profiling hook (the container's antenv stub
    lacks axon_hooks, so trn_boot's registration was skipped). Also keep
    profile artifacts local instead of uploading to a share."""
    import types

    import antenv

    if "antenv.axon_hooks" not in sys.modules:
        mod = types.ModuleType("antenv.axon_hooks")
        holder = [None]
        mod.set_axon_ntff_profile_hook = lambda h: holder.__setitem__(0, h)
        mod.get_axon_ntff_profile_hook = lambda: holder[0]
        sys.modules["antenv.axon_hooks"] = mod
        antenv.axon_hooks = mod
    from antenv import axon_hooks

    if axon_hooks.get_axon_ntff_profile_hook() is None:
        from trn_agent_boot.trn_boot import _ntff_profile_via_ctypes

        axon_hooks.set_axon_ntff_profile_hook(
            _ntff_profile_via_ctypes("/opt/axon/libaxon_pjrt.so")
        )
    bass_utils.upload_artifacts = lambda tmpdir: tmpdir


def _get_nc():
    if "nc" not in _CACHE:
        _CACHE["nc"] = _build()
    return _CACHE["nc"]


def kernel(x_real: np.ndarray, x_fake: np.ndarray) -> np.ndarray:
    global LAST_EXEC_NS
    nc = _get_nc()

    in_maps = []
    for c in range(NCORES):
        sl = slice(c * BPC, (c + 1) * BPC)
        in_maps.append(
            {
                "x_real": np.ascontiguousarray(x_real[sl]).reshape(BPC * C * P, F),
                "x_fake": np.ascontiguousarray(x_fake[sl]).reshape(BPC * C * P, F),
            }
        )

    import os

    trace = bool(int(os.environ.get("KERNEL_TRACE", "0")))
    if trace:
        _register_ntff_hook()
    res = bass_utils.run_bass_kernel_spmd(
        nc, in_maps, core_ids=list(range(NCORES)), trace=trace
    )
    LAST_EXEC_NS = res.exec_time_ns
    _CACHE["last_res"] = res

    # Decode.  Packed hue cols are exact ints in f32: split via % and //.
    # Sign-sum cols decode as N_lt = (N - S)/2.  Sat counts come from the
    # PE PSUM accumulators (already plain cumulative counts).
    PK = int(PACK)
    hue_lo = np.zeros((2, 4))      # A,B,C,D
    hue_hi = np.zeros((2, 4))      # F,G,H,I tilde counts (u<0 side)
    E_sign = np.zeros(2)
    sign_sums = np.zeros((2, 9))
    C_sat = np.zeros((2, 9))
    C_val05 = np.zeros(2)
    for core_out in res.results:
        o = np.asarray(core_out["out"]).reshape(NITER, P, ACCW).astype(np.int64)
        s = np.asarray(core_out["sat"]).astype(np.int64)
        C_sat[0] += s[0:NSAT, 0]
        C_sat[1] += s[32 : 32 + NSAT, 1]
        C_val05[0] += s[NSAT, 0]
        C_val05[1] += s[32 + NSAT, 1]
        for t, sl in ((0, slice(0, BPC)), (1, slice(BPC, NITER))):
            blk = o[sl]
            packed = blk[:, :, 0:4]
            hue_lo[t] += (packed % PK).sum(axis=(0, 1))
            hue_hi[t] += (packed // PK).sum(axis=(0, 1))
            # sign-sums live in the odd iterations of each half
            sign_sums[t] += blk[(1, 3), :, 10:19].sum(axis=(0, 1))
            E_sign[t] += blk[(1, 3), :, 4].sum() + blk[(1, 3), :, 5].sum()

    # hue cumulative counts [2, 9]
    E = (NPIX + E_sign) / 2.0
    C_hue = np.stack([
        hue_lo[:, 0], hue_lo[:, 1], hue_lo[:, 2], hue_lo[:, 3], E,
        NPIX - hue_hi[:, 0], NPIX - hue_hi[:, 1],
        NPIX - hue_hi[:, 2], NPIX - hue_hi[:, 3],
    ], axis=1)
    C_val = (NPIX - sign_sums) / 2.0
    C_val[:, 4] = C_val05

    loss = 0.0
    for wgt, Cc in ((ALPHA, C_hue), (BETA, C_sat), (GAMMA, C_val)):
        hist = np.zeros((2, 10))
        hist[:, 0] = Cc[:, 0]
        hist[:, 1:9] = Cc[:, 1:] - Cc[:, :-1]
        hist[:, 9] = NPIX - Cc[:, 8]
        loss += wgt * np.abs(hist[0] - hist[1]).mean()
    return np.asarray(loss, dtype=np.float32)


# revision 20
# speedup vs baseline: 1.0399x; 1.0047x over previous
"""ColorHistogramLoss Trainium2 kernel (8 NeuronCores, data-parallel).

Strategy: shard batch (32 -> 4 per core); each core streams its 25MB of
pixels through SBUF as 8 iterations of [128, 2048] plane-triples (4 real +
4 fake).  Counting is split across four engines so the DVE (the old
bottleneck) only carries what no other engine can:

- hue (9 edges): geometric ray tests on (u=g-b, v=b-r) as 4 dual-packed
  f32 custom-DVE ops (antipodal ray pairs share a boundary line; the
  sign(u) gate routes counts into a cnt + 4096*cnt packed accumulator)
  plus an E = #[u>=0] sign-sum on ScalarE.  f32 because the custom-DVE
  uop pipeline miscomputes on fp16 operands (measured).
- sat (9 edges): sat < c  <=>  mn/mx > 1-c.  ScalarE computes
  rmx = reciprocal(mx16); VectorE forms ratio16 = mn16 * rmx and then
  nine fp16 tensor_scalar indicator tiles (no accum -> 4x DVE mode,
  ~0.6us each); the TENSOR engine colsums each indicator via one-hot
  stationaries into a PSUM accumulator (216ns per 512-chunk, running
  concurrently with the DVE at no cost), accumulated over all 8
  iterations; one DVE reduce at the end reads the 18 totals.
- val (9 edges): Sign activations on fp16 mx pairs on ScalarE with fused
  accumulation (host decodes N_lt = (N - sign_sum)/2), batched over
  iteration pairs to amortize the fixed activation cost.
- min/max chains run in fp16 on the DVE at 2x (r16/g16/b16 casts ride
  the ScalarE Copy activation; Sign/Reciprocal/Copy share one act table
  so there is a single table load).

All counts are exact integers; only boundary-ulp pixels (fp16 rounding
of mx/mn/ratio) differ from the f32 reference (rel err ~6e-3 measured
against the reference loss, gate is 2e-2).
"""

import sys

if "/opt/trn_rl_repo" not in sys.path:
    sys.path.insert(0, "/opt/trn_rl_repo")

import numpy as np

from concourse import bacc, mybir, tile
from concourse import bass_utils

# ---- problem constants (hardcoded; kernel.py must be self-contained) ----
B, C, H, W = 32, 3, 512, 512
NCORES = 8
BPC = B // NCORES            # batches per core
P, F = 128, 2048             # SBUF tile: one [512,512] plane = [128, 2048]
NITER = 2 * BPC              # 4 real + 4 fake plane-triple iterations
ACCW = 20                    # accumulator columns per iteration
NPIX = B * H * W             # pixels per full histogram
ALPHA, BETA, GAMMA = 0.3, 0.4, 0.4

AF = mybir.AluOpType
F32 = mybir.dt.float32
F16 = mybir.dt.float16

LAST_EXEC_NS = None
_CACHE = {}

PACK = 4096.0  # dual-count packing: accum = cntA + PACK*cntB (exact in f32)
C23 = float(np.float32(2.0) / np.float32(3.0))

SVAL_EDGES = (0.1, 0.2, 0.3, 0.4, 0.5, 0.6, 0.7, 0.8, 0.9)
SAT_EDGES = (0.1, 0.2, 0.3, 0.4, 0.5, 0.6, 0.7, 0.8, 0.9)
NSAT = len(SAT_EDGES)
NK = NSAT + 2                 # PE psum rows: 9 sat + val-0.5 + val-0.4


def _register_custom_ops():
    """Author + register fused DVE ops in the dve_ops registry at runtime
    (the repo list is read-only; registration is by-name so appending to the
    module-level OPS list is sufficient for table-gen and tracing)."""
    from concourse import dve_ops
    from concourse.dve_spec import (
        C0, C1, C2, Spec, Src0, Src1, Zero, One, _has_src1, lower, maxx,
        minn, select,
    )
    from concourse.dve_uop import DveOpSpec

    if hasattr(dve_ops, "HPA"):
        return dve_ops

    from operator import add as _add

    def _accref(body_fn):
        def ref(in0, in1, c0, c1, c2):
            b = body_fn(
                np.asarray(in0, np.float32),
                np.asarray(in1, np.float32) if in1 is not None else None,
                c0, c1, c2,
            ).astype(np.float32)
            return b, b.reshape(b.shape[0], -1).sum(axis=-1, keepdims=True)
        return ref

    # gate: 1 where u>=0 else PACK (routes the count into the high field)
    gate = select(Src0 >= Zero, One, C1)

    defs = [
        # hue pair, A-form: t = (Src0 + C0*Src1 <= 0); accum t*(1|C1 by sign)
        (
            "HPA",
            Spec(
                body=((Src0 + C0 * Src1) <= Zero) * gate,
                accum=_add,
                accum_init=Zero,
                reference=_accref(
                    lambda u, v, c0, c1, c2: ((u + np.float32(c0) * v) <= 0)
                    * np.where(u >= 0, 1.0, c1)
                ),
            ),
        ),
        # hue pair, B-form: t = (Src1 + C0*Src0 <= 0)
        (
            "HPB",
            Spec(
                body=((Src1 + C0 * Src0) <= Zero) * gate,
                accum=_add,
                accum_init=Zero,
                reference=_accref(
                    lambda u, v, c0, c1, c2: ((v + np.float32(c0) * u) <= 0)
                    * np.where(u >= 0, 1.0, c1)
                ),
            ),
        ),
    ]
    for name, spec in defs:
        row = 1 + len(dve_ops.OPS)
        shas = {}
        for ver in ("v3", "v4"):
            uops = lower(spec, ver=ver)
            shas[ver] = DveOpSpec(
                name=name, opcode=row, uops=uops, rd1_en=_has_src1(spec)
            ).sha(ver)
        op = dve_ops.DveOp(name, spec, False, uops_sha=shas)
        dve_ops.OPS.append(op)
        dve_ops.CUSTOM_DVE_SPECS[name] = spec
        dve_ops._SUB_OPCODE_FOR_NAME[name] = row
        setattr(dve_ops, name, op)
    return dve_ops


def _build():
    dve_ops = _register_custom_ops()
    nc = bacc.Bacc(
        "TRN2", target_bir_lowering=False, debug=False, num_devices=NCORES
    )
    xr = nc.dram_tensor("x_real", [BPC * C * P, F], F32, kind="ExternalInput").ap()
    xf = nc.dram_tensor("x_fake", [BPC * C * P, F], F32, kind="ExternalInput").ap()
    out = nc.dram_tensor("out", [NITER * P, ACCW], F32, kind="ExternalOutput").ap()
    sat_out = nc.dram_tensor("sat", [P, 2], F32, kind="ExternalOutput").ap()

    SIGN = mybir.ActivationFunctionType.Sign
    COPY = mybir.ActivationFunctionType.Copy
    LN = mybir.ActivationFunctionType.Ln

    with tile.TileContext(nc) as tc:
        with tc.tile_pool(name="main", bufs=2) as io_pool, tc.tile_pool(
            name="tmp", bufs=1
        ) as tmp_pool, tc.tile_pool(name="ps", bufs=1, space="PSUM") as ppool:
            # per-edge bias tiles for ScalarE Sign activations (bias = -edge)
            sbias = []
            for e in SVAL_EDGES + (0.0,):       # 0.0: E-count sign(u)
                bt = tmp_pool.tile([P, 1], F32, tag=f"sb{int(e*10)}",
                                   name=f"sb{int(e*10)}")
                nc.gpsimd.memset(bt[:], -e)
                sbias.append(bt)
            # one-hot stationaries for the PE colsum (col j = ones)
            stats = []
            for j in range(NK):
                st = tmp_pool.tile([P, NK], F16, tag=f"st{j}", name=f"st{j}")
                nc.vector.memset(st[:], 0.0)
                nc.vector.memset(st[:, j : j + 1], 1.0)
                stats.append(st)
            psum = ppool.tile([P, F], F32, tag="psum")
            scr2b = tmp_pool.tile([P, 2, F], F16, tag="scr2b", name="scr2b")
            ind_rot = [
                tmp_pool.tile([P, F], F16, tag=f"ind{k}", name=f"ind{k}")
                for k in range(4)
            ]
            acc_sat = tmp_pool.tile([P, 2], F32, tag="acc_sat", name="acc_sat")
            nc.vector.memset(acc_sat[:], 0.0)
            # f32 scratch shared by the custom-DVE dummy outs and the final
            # PSUM readback
            scr = tmp_pool.tile([P, F], F32, tag="scr", name="scr")

            V = nc.vector
            S = nc.scalar
            for it in range(NITER):
                src = xr if it < BPC else xf
                bi = it % BPC

                def plane(c):
                    q = bi * C + c
                    return src[q * P : (q + 1) * P, :]

                r = io_pool.tile([P, F], F32, tag="r")
                g = io_pool.tile([P, F], F32, tag="g")
                bl = io_pool.tile([P, F], F32, tag="bl")
                nc.sync.dma_start(g[:], plane(1))
                nc.sync.dma_start(bl[:], plane(2))
                nc.sync.dma_start(r[:], plane(0))

                # u and mx16 live in [P, 2, F] pair tiles so ScalarE can run
                # one batched activation over two iterations' data
                if it % 2 == 0:
                    upair = io_pool.tile([P, 2, F], F32, tag="upair")
                    mxpair = io_pool.tile([P, 2, F], F16, tag="mxpair")
                u = upair[:, it % 2, :]
                mx16 = mxpair[:, it % 2, :]
                v = tmp_pool.tile([P, F], F32, tag="v", name="v")
                r16 = io_pool.tile([P, F], F16, tag="r16")
                g16 = io_pool.tile([P, F], F16, tag="g16")
                b16 = io_pool.tile([P, F], F16, tag="b16")
                m1 = tmp_pool.tile([P, F], F16, tag="m1", name="m1")
                mn1 = tmp_pool.tile([P, F], F16, tag="mn1", name="mn1")
                mn16 = io_pool.tile([P, F], F16, tag="mn16")
                rmx = io_pool.tile([P, F], F16, tag="rmx")
                ratio = io_pool.tile([P, F], F16, tag="ratio")
                acc = io_pool.tile([P, ACCW], F32, tag="acc")

                # ScalarE: fp16 casts of the three planes
                S.activation(g16[:], g[:], COPY)
                S.activation(b16[:], bl[:], COPY)
                S.activation(r16[:], r[:], COPY)

                # DVE: u, v in f32 (custom-DVE hue ops need f32 operands)
                V.tensor_tensor(u, g[:], bl[:], AF.subtract)
                V.tensor_tensor(v[:], bl[:], r[:], AF.subtract)
                # fp16 min/max chains first (2x DVE mode): ScalarE's Ln and
                # Sign work hangs off mx16/mn16, so feed it before the long
                # hue-custom block
                V.tensor_tensor(m1[:], r16[:], g16[:], AF.max)
                V.tensor_tensor(mx16, m1[:], b16[:], AF.max)
                V.tensor_tensor(mn1[:], r16[:], g16[:], AF.min)
                V.tensor_tensor(mn16[:], mn1[:], b16[:], AF.min)
                # hue pairs on (u, v): acc0..3
                V._custom_dve(dve_ops.HPA, out=scr[:], in0=u, in1=v[:],
                              s0=0.6, s1=PACK, accum_out=acc[:, 0:1])
                V._custom_dve(dve_ops.HPB, out=scr[:], in0=u, in1=v[:],
                              s0=0.8, s1=PACK, accum_out=acc[:, 1:2])
                V._custom_dve(dve_ops.HPB, out=scr[:], in0=u, in1=v[:],
                              s0=0.2, s1=PACK, accum_out=acc[:, 2:3])
                V._custom_dve(dve_ops.HPB, out=scr[:], in0=u, in1=v[:],
                              s0=-C23, s1=PACK, accum_out=acc[:, 3:4])
                # sat < c  <=>  mn/mx > 1-c  <=>  ln(mn) - ln(mx) > ln(1-c).
                # (ScalarE Reciprocal is blocked for accuracy; Ln shares the
                # natural_log act table with Sign and Copy -> no reloads.
                # ln(0) = -inf keeps the mn==0 pixels in the right bin.)
                S.activation(rmx[:], mx16, LN)
                S.activation(ratio[:], mn16[:], LN)
                w = tmp_pool.tile([P, F], F16, tag="w", name="w")
                V.tensor_tensor(w[:], ratio[:], rmx[:], AF.subtract)
                # sat indicators (4x DVE mode, no accum) + PE colsum into PSUM
                base = 0 if it < BPC else 32
                rows = slice(base, base + NK)
                for j, c in enumerate(SAT_EDGES):
                    indt = ind_rot[j % 4][:]
                    V.tensor_scalar(indt, w[:], float(np.log(1.0 - c)), None,
                                    AF.is_gt)
                    for cj in range(4):
                        nc.tensor.matmul(
                            psum[rows, cj * 512 : (cj + 1) * 512],
                            stats[j][:, :],
                            indt[:, cj * 512 : (cj + 1) * 512],
                            start=(it % BPC == 0 and j == 0),
                            stop=False,
                        )

                # val-0.5 and val-0.4 ride the PE path too (ScalarE -> PE)
                for vj, ve in ((0, 0.5), (1, 0.4)):
                    indt = ind_rot[(NSAT + vj) % 4][:]
                    V.tensor_scalar(indt, mx16, ve, None, AF.is_lt)
                    for cj in range(4):
                        nc.tensor.matmul(
                            psum[rows, cj * 512 : (cj + 1) * 512],
                            stats[NSAT + vj][:, :],
                            indt[:, cj * 512 : (cj + 1) * 512],
                            start=False,
                            stop=(it % BPC == BPC - 1 and vj == 1),
                        )

                if it % 2 == 1:
                    # ScalarE, batched over the iteration pair: val-9 + E
                    # acc10..18 (val sign-sums over 2*F on fp16 mx), acc4 (E)
                    for k in range(9):
                        if k in (3, 4):  # 0.4/0.5 are counted on the PE path
                            continue
                        S.activation(
                            scr2b[:], mxpair[:], SIGN, bias=sbias[k][:],
                            accum_out=acc[:, 10 + k : 11 + k],
                        )
                    S.activation(
                        scr2b[:], upair[:], SIGN,
                        bias=sbias[9][:], accum_out=acc[:, 4:5],
                    )
                nc.sync.dma_start(out[it * P : (it + 1) * P, :], acc[:, :])

            # final: read the PE sat accumulators out of PSUM
            V.tensor_scalar(scr[0:NK, :], psum[0:NK, :], 1.0, None,
                            AF.mult, AF.add, accum_out=acc_sat[0:NK, 0:1])
            V.tensor_scalar(scr[0:NK, :], psum[32 : 32 + NK, :], 1.0,
                            None, AF.mult, AF.add,
                            accum_out=acc_sat[32 : 32 + NK, 1:2])
            nc.sync.dma_start(sat_out, acc_sat[:])

    nc.compile()
    return nc


def _register_ntff_hook():
    """Register the axon NTFF profiling hook (the container's antenv stub
    lacks axon_hooks, so trn_boot's registration was skipped). Also keep
    profile artifacts local instead of uploading to a share."""
    import types

    import antenv

    if "antenv.axon_hooks" not in sys.modules:
        mod = types.ModuleType("antenv.axon_hooks")
        holder = [None]
        mod.set_axon_ntff_profile_hook = lambda h: holder.__setitem__(0, h)
        mod.get_axon_ntff_profile_hook = lambda: holder[0]
        sys.modules["antenv.axon_hooks"] = mod
        antenv.axon_hooks = mod
    from antenv import axon_hooks

    if axon_hooks.get_axon_ntff_profile_hook() is None:
        from trn_agent_boot.trn_boot import _ntff_profile_via_ctypes

        axon_hooks.set_axon_ntff_profile_hook(
            _ntff_profile_via_ctypes("/opt/axon/libaxon_pjrt.so")
        )
    bass_utils.upload_artifacts = lambda tmpdir: tmpdir


def _get_nc():
    if "nc" not in _CACHE:
        _CACHE["nc"] = _build()
    return _CACHE["nc"]


def kernel(x_real: np.ndarray, x_fake: np.ndarray) -> np.ndarray:
    global LAST_EXEC_NS
    nc = _get_nc()

    in_maps = []
    for c in range(NCORES):
        sl = slice(c * BPC, (c + 1) * BPC)
        in_maps.append(
            {
                "x_real": np.ascontiguousarray(x_real[sl]).reshape(BPC * C * P, F),
                "x_fake": np.ascontiguousarray(x_fake[sl]).reshape(BPC * C * P, F),
            }
        )

    import os

    trace = bool(int(os.environ.get("KERNEL_TRACE", "0")))
    if trace:
        _register_ntff_hook()
    res = bass_utils.run_bass_kernel_spmd(
        nc, in_maps, core_ids=list(range(NCORES)), trace=trace
    )
    LAST_EXEC_NS = res.exec_time_ns
    _CACHE["last_res"] = res

    # Decode.  Packed hue cols are exact ints in f32: split via % and //.
    # Sign-sum cols decode as N_lt = (N - S)/2.  Sat counts come from the
    # PE PSUM accumulators (already plain cumulative counts).
    PK = int(PACK)
    hue_lo = np.zeros((2, 4))      # A,B,C,D
    hue_hi = np.zeros((2, 4))      # F,G,H,I tilde counts (u<0 side)
    E_sign = np.zeros(2)
    sign_sums = np.zeros((2, 9))
    C_sat = np.zeros((2, 9))
    C_val05 = np.zeros(2)
    C_val04 = np.zeros(2)
    for core_out in res.results:
        o = np.asarray(core_out["out"]).reshape(NITER, P, ACCW).astype(np.int64)
        s = np.asarray(core_out["sat"]).astype(np.int64)
        C_sat[0] += s[0:NSAT, 0]
        C_sat[1] += s[32 : 32 + NSAT, 1]
        C_val05[0] += s[NSAT, 0]
        C_val05[1] += s[32 + NSAT, 1]
        C_val04[0] += s[NSAT + 1, 0]
        C_val04[1] += s[32 + NSAT + 1, 1]
        for t, sl in ((0, slice(0, BPC)), (1, slice(BPC, NITER))):
            blk = o[sl]
            packed = blk[:, :, 0:4]
            hue_lo[t] += (packed % PK).sum(axis=(0, 1))
            hue_hi[t] += (packed // PK).sum(axis=(0, 1))
            # sign-sums live in the odd iterations of each half
            sign_sums[t] += blk[(1, 3), :, 10:19].sum(axis=(0, 1))
            E_sign[t] += blk[(1, 3), :, 4].sum() + blk[(1, 3), :, 5].sum()

    # hue cumulative counts [2, 9]
    E = (NPIX + E_sign) / 2.0
    C_hue = np.stack([
        hue_lo[:, 0], hue_lo[:, 1], hue_lo[:, 2], hue_lo[:, 3], E,
        NPIX - hue_hi[:, 0], NPIX - hue_hi[:, 1],
        NPIX - hue_hi[:, 2], NPIX - hue_hi[:, 3],
    ], axis=1)
    C_val = (NPIX - sign_sums) / 2.0
    C_val[:, 4] = C_val05
    C_val[:, 3] = C_val04

    loss = 0.0
    for wgt, Cc in ((ALPHA, C_hue), (BETA, C_sat), (GAMMA, C_val)):
        hist = np.zeros((2, 10))
        hist[:, 0] = Cc[:, 0]
        hist[:, 1:9] = Cc[:, 1:] - Cc[:, :-1]
        hist[:, 9] = NPIX - Cc[:, 8]
        loss += wgt * np.abs(hist[0] - hist[1]).mean()
    return np.asarray(loss, dtype=np.float32)


# revision 22
# speedup vs baseline: 1.0744x; 1.0332x over previous
"""ColorHistogramLoss Trainium2 kernel (8 NeuronCores, data-parallel).

Strategy: shard batch (32 -> 4 per core); each core streams its 25MB of
pixels through SBUF as 8 iterations of [128, 2048] plane-triples (4 real +
4 fake).  Counting is split across four engines so the DVE (the old
bottleneck) only carries what no other engine can:

- hue (9 edges): geometric ray tests on (u=g-b, v=b-r) as 4 dual-packed
  f32 custom-DVE ops (antipodal ray pairs share a boundary line; the
  sign(u) gate routes counts into a cnt + 4096*cnt packed accumulator)
  plus an E = #[u>=0] sign-sum on ScalarE.  f32 because the custom-DVE
  uop pipeline miscomputes on fp16 operands (measured).
- sat (9 edges): sat < c  <=>  mn/mx > 1-c.  ScalarE computes
  rmx = reciprocal(mx16); VectorE forms ratio16 = mn16 * rmx and then
  nine fp16 tensor_scalar indicator tiles (no accum -> 4x DVE mode,
  ~0.6us each); the TENSOR engine colsums each indicator via one-hot
  stationaries into a PSUM accumulator (216ns per 512-chunk, running
  concurrently with the DVE at no cost), accumulated over all 8
  iterations; one DVE reduce at the end reads the 18 totals.
- val (9 edges): Sign activations on fp16 mx pairs on ScalarE with fused
  accumulation (host decodes N_lt = (N - sign_sum)/2), batched over
  iteration pairs to amortize the fixed activation cost.
- min/max chains run in fp16 on the DVE at 2x (r16/g16/b16 casts ride
  the ScalarE Copy activation; Sign/Reciprocal/Copy share one act table
  so there is a single table load).

All counts are exact integers; only boundary-ulp pixels (fp16 rounding
of mx/mn/ratio) differ from the f32 reference (rel err ~6e-3 measured
against the reference loss, gate is 2e-2).
"""

import sys

if "/opt/trn_rl_repo" not in sys.path:
    sys.path.insert(0, "/opt/trn_rl_repo")

import numpy as np

from concourse import bacc, mybir, tile
from concourse import bass_utils

# ---- problem constants (hardcoded; kernel.py must be self-contained) ----
B, C, H, W = 32, 3, 512, 512
NCORES = 8
BPC = B // NCORES            # batches per core
P, F = 128, 2048             # SBUF tile: one [512,512] plane = [128, 2048]
NITER = 2 * BPC              # 4 real + 4 fake plane-triple iterations
ACCW = 20                    # accumulator columns per iteration
NPIX = B * H * W             # pixels per full histogram
ALPHA, BETA, GAMMA = 0.3, 0.4, 0.4

AF = mybir.AluOpType
F32 = mybir.dt.float32
F16 = mybir.dt.float16

LAST_EXEC_NS = None
_CACHE = {}

PACK = 4096.0  # dual-count packing: accum = cntA + PACK*cntB (exact in f32)
C23 = float(np.float32(2.0) / np.float32(3.0))

SVAL_EDGES = (0.1, 0.2, 0.3, 0.4, 0.5, 0.6, 0.7, 0.8, 0.9)
SAT_EDGES = (0.1, 0.2, 0.3, 0.4, 0.5, 0.6, 0.7, 0.8, 0.9)
NSAT = len(SAT_EDGES)
NK = NSAT + 2                 # PE psum rows: 9 sat + val-0.5 + val-0.4


def _register_custom_ops():
    """Author + register fused DVE ops in the dve_ops registry at runtime
    (the repo list is read-only; registration is by-name so appending to the
    module-level OPS list is sufficient for table-gen and tracing)."""
    from concourse import dve_ops
    from concourse.dve_spec import (
        C0, C1, C2, Spec, Src0, Src1, Zero, One, _has_src1, lower, maxx,
        minn, select,
    )
    from concourse.dve_uop import DveOpSpec

    if hasattr(dve_ops, "HPA"):
        return dve_ops

    from operator import add as _add

    def _accref(body_fn):
        def ref(in0, in1, c0, c1, c2):
            b = body_fn(
                np.asarray(in0, np.float32),
                np.asarray(in1, np.float32) if in1 is not None else None,
                c0, c1, c2,
            ).astype(np.float32)
            return b, b.reshape(b.shape[0], -1).sum(axis=-1, keepdims=True)
        return ref

    # gate: 1 where u>=0 else PACK (routes the count into the high field)
    gate = select(Src0 >= Zero, One, C1)

    defs = [
        # hue pair, A-form: t = (Src0 + C0*Src1 <= 0); accum t*(1|C1 by sign)
        (
            "HPA",
            Spec(
                body=((Src0 + C0 * Src1) <= Zero) * gate,
                accum=_add,
                accum_init=Zero,
                reference=_accref(
                    lambda u, v, c0, c1, c2: ((u + np.float32(c0) * v) <= 0)
                    * np.where(u >= 0, 1.0, c1)
                ),
            ),
        ),
        # hue pair, B-form: t = (Src1 + C0*Src0 <= 0)
        (
            "HPB",
            Spec(
                body=((Src1 + C0 * Src0) <= Zero) * gate,
                accum=_add,
                accum_init=Zero,
                reference=_accref(
                    lambda u, v, c0, c1, c2: ((v + np.float32(c0) * u) <= 0)
                    * np.where(u >= 0, 1.0, c1)
                ),
            ),
        ),
    ]
    for name, spec in defs:
        row = 1 + len(dve_ops.OPS)
        shas = {}
        for ver in ("v3", "v4"):
            uops = lower(spec, ver=ver)
            shas[ver] = DveOpSpec(
                name=name, opcode=row, uops=uops, rd1_en=_has_src1(spec)
            ).sha(ver)
        op = dve_ops.DveOp(name, spec, False, uops_sha=shas)
        dve_ops.OPS.append(op)
        dve_ops.CUSTOM_DVE_SPECS[name] = spec
        dve_ops._SUB_OPCODE_FOR_NAME[name] = row
        setattr(dve_ops, name, op)
    return dve_ops


def _build():
    dve_ops = _register_custom_ops()
    nc = bacc.Bacc(
        "TRN2", target_bir_lowering=False, debug=False, num_devices=NCORES
    )
    xr = nc.dram_tensor("x_real", [BPC * C * P, F], F32, kind="ExternalInput").ap()
    xf = nc.dram_tensor("x_fake", [BPC * C * P, F], F32, kind="ExternalInput").ap()
    out = nc.dram_tensor("out", [NITER * P, ACCW], F32, kind="ExternalOutput").ap()
    sat_out = nc.dram_tensor("sat", [P, 2], F32, kind="ExternalOutput").ap()

    SIGN = mybir.ActivationFunctionType.Sign
    COPY = mybir.ActivationFunctionType.Copy
    LN = mybir.ActivationFunctionType.Ln

    with tile.TileContext(nc) as tc:
        with tc.tile_pool(name="main", bufs=2) as io_pool, tc.tile_pool(
            name="tmp", bufs=1
        ) as tmp_pool, tc.tile_pool(name="ps", bufs=1, space="PSUM") as ppool:
            # per-edge bias tiles for ScalarE Sign activations (bias = -edge)
            sbias = []
            for e in SVAL_EDGES + (0.0,):       # 0.0: E-count sign(u)
                bt = tmp_pool.tile([P, 1], F32, tag=f"sb{int(e*10)}",
                                   name=f"sb{int(e*10)}")
                nc.gpsimd.memset(bt[:], -e)
                sbias.append(bt)
            # one-hot stationaries for the PE colsum (col j = ones)
            stats = []
            for j in range(NK):
                st = tmp_pool.tile([P, NK], F16, tag=f"st{j}", name=f"st{j}")
                nc.vector.memset(st[:], 0.0)
                nc.vector.memset(st[:, j : j + 1], 1.0)
                stats.append(st)
            psum = ppool.tile([P, F], F32, tag="psum")
            scr2b = tmp_pool.tile([P, 2, F], F16, tag="scr2b", name="scr2b")
            ind_rot = [
                tmp_pool.tile([P, 2, F], F16, tag=f"ind{k}", name=f"ind{k}")
                for k in range(2)
            ]
            acc_sat = tmp_pool.tile([P, 2], F32, tag="acc_sat", name="acc_sat")
            nc.vector.memset(acc_sat[:], 0.0)
            # f32 scratch shared by the custom-DVE dummy outs and the final
            # PSUM readback
            scr = tmp_pool.tile([P, F], F32, tag="scr", name="scr")

            V = nc.vector
            S = nc.scalar
            for it in range(NITER):
                src = xr if it < BPC else xf
                bi = it % BPC

                def plane(c):
                    q = bi * C + c
                    return src[q * P : (q + 1) * P, :]

                r = io_pool.tile([P, F], F32, tag="r")
                g = io_pool.tile([P, F], F32, tag="g")
                bl = io_pool.tile([P, F], F32, tag="bl")
                nc.sync.dma_start(g[:], plane(1))
                nc.sync.dma_start(bl[:], plane(2))
                nc.sync.dma_start(r[:], plane(0))

                # u and mx16 live in [P, 2, F] pair tiles so ScalarE can run
                # one batched activation over two iterations' data
                if it % 2 == 0:
                    upair = io_pool.tile([P, 2, F], F32, tag="upair")
                    mxpair = io_pool.tile([P, 2, F], F16, tag="mxpair")
                    wpair = io_pool.tile([P, 2, F], F16, tag="wpair")
                u = upair[:, it % 2, :]
                mx16 = mxpair[:, it % 2, :]
                v = tmp_pool.tile([P, F], F32, tag="v", name="v")
                r16 = io_pool.tile([P, F], F16, tag="r16")
                g16 = io_pool.tile([P, F], F16, tag="g16")
                b16 = io_pool.tile([P, F], F16, tag="b16")
                m1 = tmp_pool.tile([P, F], F16, tag="m1", name="m1")
                mn1 = m1
                mn16 = io_pool.tile([P, F], F16, tag="mn16")
                rmx = io_pool.tile([P, F], F16, tag="rmx")
                ratio = io_pool.tile([P, F], F16, tag="ratio")
                acc = io_pool.tile([P, ACCW], F32, tag="acc")

                # ScalarE: fp16 casts of the three planes
                S.activation(g16[:], g[:], COPY)
                S.activation(b16[:], bl[:], COPY)
                S.activation(r16[:], r[:], COPY)

                # DVE: u, v in f32 (custom-DVE hue ops need f32 operands)
                V.tensor_tensor(u, g[:], bl[:], AF.subtract)
                V.tensor_tensor(v[:], bl[:], r[:], AF.subtract)
                # fp16 min/max chains first (2x DVE mode): ScalarE's Ln and
                # Sign work hangs off mx16/mn16, so feed it before the long
                # hue-custom block
                V.tensor_tensor(m1[:], r16[:], g16[:], AF.max)
                V.tensor_tensor(mx16, m1[:], b16[:], AF.max)
                V.tensor_tensor(mn1[:], r16[:], g16[:], AF.min)
                V.tensor_tensor(mn16[:], mn1[:], b16[:], AF.min)
                # hue pairs on (u, v): acc0..3
                V._custom_dve(dve_ops.HPA, out=scr[:], in0=u, in1=v[:],
                              s0=0.6, s1=PACK, accum_out=acc[:, 0:1])
                V._custom_dve(dve_ops.HPB, out=scr[:], in0=u, in1=v[:],
                              s0=0.8, s1=PACK, accum_out=acc[:, 1:2])
                V._custom_dve(dve_ops.HPB, out=scr[:], in0=u, in1=v[:],
                              s0=0.2, s1=PACK, accum_out=acc[:, 2:3])
                V._custom_dve(dve_ops.HPB, out=scr[:], in0=u, in1=v[:],
                              s0=-C23, s1=PACK, accum_out=acc[:, 3:4])
                # sat < c  <=>  mn/mx > 1-c  <=>  ln(mn) - ln(mx) > ln(1-c).
                # (ScalarE Reciprocal is blocked for accuracy; Ln shares the
                # natural_log act table with Sign and Copy -> no reloads.
                # ln(0) = -inf keeps the mn==0 pixels in the right bin.)
                S.activation(rmx[:], mx16, LN)
                S.activation(ratio[:], mn16[:], LN)
                w = wpair[:, it % 2, :]
                V.tensor_tensor(w, ratio[:], rmx[:], AF.subtract)
                # sat indicators (4x DVE mode, no accum) + PE colsum into
                # PSUM, batched over the iteration pair
                base = 0 if it < BPC else 32
                rows = slice(base, base + NK)
                if it % 2 == 1:
                    for j, c in enumerate(SAT_EDGES):
                        indt = ind_rot[j % 2]
                        V.tensor_scalar(indt[:], wpair[:],
                                        float(np.log(1.0 - c)), None,
                                        AF.is_gt)
                        for sl in range(2):
                            for cj in range(4):
                                nc.tensor.matmul(
                                    psum[rows, cj * 512 : (cj + 1) * 512],
                                    stats[j][:, :],
                                    indt[:, sl, cj * 512 : (cj + 1) * 512],
                                    start=False,
                                    stop=False,
                                )

                # val-0.5 and val-0.4 ride the PE path too (ScalarE -> PE).
                # The half's first val matmuls carry start=True, zeroing all
                # NK psum rows of the region (one-hot stationary writes 0s
                # to the other rows).
                for vj, ve in ((0, 0.5), (1, 0.4)):
                    indt = ind_rot[(NSAT + vj) % 2]
                    V.tensor_scalar(indt[:, 0, :], mx16, ve, None, AF.is_lt)
                    for cj in range(4):
                        nc.tensor.matmul(
                            psum[rows, cj * 512 : (cj + 1) * 512],
                            stats[NSAT + vj][:, :],
                            indt[:, 0, cj * 512 : (cj + 1) * 512],
                            start=(it % BPC == 0 and vj == 0),
                            stop=(it % BPC == BPC - 1 and vj == 1),
                        )

                if it % 2 == 1:
                    # ScalarE, batched over the iteration pair: val-9 + E
                    # acc10..18 (val sign-sums over 2*F on fp16 mx), acc4 (E)
                    for k in range(9):
                        if k in (3, 4):  # 0.4/0.5 are counted on the PE path
                            continue
                        S.activation(
                            scr2b[:], mxpair[:], SIGN, bias=sbias[k][:],
                            accum_out=acc[:, 10 + k : 11 + k],
                        )
                    S.activation(
                        scr2b[:], upair[:], SIGN,
                        bias=sbias[9][:], accum_out=acc[:, 4:5],
                    )
                nc.sync.dma_start(out[it * P : (it + 1) * P, :], acc[:, :])

            # final: read the PE sat accumulators out of PSUM
            V.tensor_scalar(scr[0:NK, :], psum[0:NK, :], 1.0, None,
                            AF.mult, AF.add, accum_out=acc_sat[0:NK, 0:1])
            V.tensor_scalar(scr[0:NK, :], psum[32 : 32 + NK, :], 1.0,
                            None, AF.mult, AF.add,
                            accum_out=acc_sat[32 : 32 + NK, 1:2])
            nc.sync.dma_start(sat_out, acc_sat[:])

    nc.compile()
    return nc


def _register_ntff_hook():
    """Register the axon NTFF profiling hook (the container's antenv stub
    lacks axon_hooks, so trn_boot's registration was skipped). Also keep
    profile artifacts local instead of uploading to a share."""
    import types

    import antenv

    if "antenv.axon_hooks" not in sys.modules:
        mod = types.ModuleType("antenv.axon_hooks")
        holder = [None]
        mod.set_axon_ntff_profile_hook = lambda h: holder.__setitem__(0, h)
        mod.get_axon_ntff_profile_hook = lambda: holder[0]
        sys.modules["antenv.axon_hooks"] = mod
        antenv.axon_hooks = mod
    from antenv import axon_hooks

    if axon_hooks.get_axon_ntff_profile_hook() is None:
        from trn_agent_boot.trn_boot import _ntff_profile_via_ctypes

        axon_hooks.set_axon_ntff_profile_hook(
            _ntff_profile_via_ctypes("/opt/axon/libaxon_pjrt.so")
        )
    bass_utils.upload_artifacts = lambda tmpdir: tmpdir


def _get_nc():
    if "nc" not in _CACHE:
        _CACHE["nc"] = _build()
    return _CACHE["nc"]


def kernel(x_real: np.ndarray, x_fake: np.ndarray) -> np.ndarray:
    global LAST_EXEC_NS
    nc = _get_nc()

    in_maps = []
    for c in range(NCORES):
        sl = slice(c * BPC, (c + 1) * BPC)
        in_maps.append(
            {
                "x_real": np.ascontiguousarray(x_real[sl]).reshape(BPC * C * P, F),
                "x_fake": np.ascontiguousarray(x_fake[sl]).reshape(BPC * C * P, F),
            }
        )

    import os

    trace = bool(int(os.environ.get("KERNEL_TRACE", "0")))
    if trace:
        _register_ntff_hook()
    res = bass_utils.run_bass_kernel_spmd(
        nc, in_maps, core_ids=list(range(NCORES)), trace=trace
    )
    LAST_EXEC_NS = res.exec_time_ns
    _CACHE["last_res"] = res

    # Decode.  Packed hue cols are exact ints in f32: split via % and //.
    # Sign-sum cols decode as N_lt = (N - S)/2.  Sat counts come from the
    # PE PSUM accumulators (already plain cumulative counts).
    PK = int(PACK)
    hue_lo = np.zeros((2, 4))      # A,B,C,D
    hue_hi = np.zeros((2, 4))      # F,G,H,I tilde counts (u<0 side)
    E_sign = np.zeros(2)
    sign_sums = np.zeros((2, 9))
    C_sat = np.zeros((2, 9))
    C_val05 = np.zeros(2)
    C_val04 = np.zeros(2)
    for core_out in res.results:
        o = np.asarray(core_out["out"]).reshape(NITER, P, ACCW).astype(np.int64)
        s = np.asarray(core_out["sat"]).astype(np.int64)
        C_sat[0] += s[0:NSAT, 0]
        C_sat[1] += s[32 : 32 + NSAT, 1]
        C_val05[0] += s[NSAT, 0]
        C_val05[1] += s[32 + NSAT, 1]
        C_val04[0] += s[NSAT + 1, 0]
        C_val04[1] += s[32 + NSAT + 1, 1]
        for t, sl in ((0, slice(0, BPC)), (1, slice(BPC, NITER))):
            blk = o[sl]
            packed = blk[:, :, 0:4]
            hue_lo[t] += (packed % PK).sum(axis=(0, 1))
            hue_hi[t] += (packed // PK).sum(axis=(0, 1))
            # sign-sums live in the odd iterations of each half
            sign_sums[t] += blk[(1, 3), :, 10:19].sum(axis=(0, 1))
            E_sign[t] += blk[(1, 3), :, 4].sum() + blk[(1, 3), :, 5].sum()

    # hue cumulative counts [2, 9]
    E = (NPIX + E_sign) / 2.0
    C_hue = np.stack([
        hue_lo[:, 0], hue_lo[:, 1], hue_lo[:, 2], hue_lo[:, 3], E,
        NPIX - hue_hi[:, 0], NPIX - hue_hi[:, 1],
        NPIX - hue_hi[:, 2], NPIX - hue_hi[:, 3],
    ], axis=1)
    C_val = (NPIX - sign_sums) / 2.0
    C_val[:, 4] = C_val05
    C_val[:, 3] = C_val04

    loss = 0.0
    for wgt, Cc in ((ALPHA, C_hue), (BETA, C_sat), (GAMMA, C_val)):
        hist = np.zeros((2, 10))
        hist[:, 0] = Cc[:, 0]
        hist[:, 1:9] = Cc[:, 1:] - Cc[:, :-1]
        hist[:, 9] = NPIX - Cc[:, 8]
        loss += wgt * np.abs(hist[0] - hist[1]).mean()
    return np.asarray(loss, dtype=np.float32)


# revision 24
# speedup vs baseline: 1.0758x; 1.0013x over previous
"""ColorHistogramLoss Trainium2 kernel (8 NeuronCores, data-parallel).

Strategy: shard batch (32 -> 4 per core); each core streams its 25MB of
pixels through SBUF as 8 iterations of [128, 2048] plane-triples (4 real +
4 fake).  Counting is split across four engines so the DVE (the old
bottleneck) only carries what no other engine can:

- hue (9 edges): geometric ray tests on (u=g-b, v=b-r) as 4 dual-packed
  f32 custom-DVE ops (antipodal ray pairs share a boundary line; the
  sign(u) gate routes counts into a cnt + 4096*cnt packed accumulator)
  plus an E = #[u>=0] sign-sum on ScalarE.  f32 because the custom-DVE
  uop pipeline miscomputes on fp16 operands (measured).
- sat (9 edges): sat < c  <=>  mn/mx > 1-c.  ScalarE computes
  rmx = reciprocal(mx16); VectorE forms ratio16 = mn16 * rmx and then
  nine fp16 tensor_scalar indicator tiles (no accum -> 4x DVE mode,
  ~0.6us each); the TENSOR engine colsums each indicator via one-hot
  stationaries into a PSUM accumulator (216ns per 512-chunk, running
  concurrently with the DVE at no cost), accumulated over all 8
  iterations; one DVE reduce at the end reads the 18 totals.
- val (9 edges): Sign activations on fp16 mx pairs on ScalarE with fused
  accumulation (host decodes N_lt = (N - sign_sum)/2), batched over
  iteration pairs to amortize the fixed activation cost.
- min/max chains run in fp16 on the DVE at 2x (r16/g16/b16 casts ride
  the ScalarE Copy activation; Sign/Reciprocal/Copy share one act table
  so there is a single table load).

All counts are exact integers; only boundary-ulp pixels (fp16 rounding
of mx/mn/ratio) differ from the f32 reference (rel err ~6e-3 measured
against the reference loss, gate is 2e-2).
"""

import sys

if "/opt/trn_rl_repo" not in sys.path:
    sys.path.insert(0, "/opt/trn_rl_repo")

import numpy as np

from concourse import bacc, mybir, tile
from concourse import bass_utils

# ---- problem constants (hardcoded; kernel.py must be self-contained) ----
B, C, H, W = 32, 3, 512, 512
NCORES = 8
BPC = B // NCORES            # batches per core
P, F = 128, 2048             # SBUF tile: one [512,512] plane = [128, 2048]
NITER = 2 * BPC              # 4 real + 4 fake plane-triple iterations
ACCW = 20                    # accumulator columns per iteration
NPIX = B * H * W             # pixels per full histogram
ALPHA, BETA, GAMMA = 0.3, 0.4, 0.4

AF = mybir.AluOpType
F32 = mybir.dt.float32
F16 = mybir.dt.float16

LAST_EXEC_NS = None
_CACHE = {}

PACK = 4096.0  # dual-count packing: accum = cntA + PACK*cntB (exact in f32)
C23 = float(np.float32(2.0) / np.float32(3.0))

SVAL_EDGES = (0.1, 0.2, 0.3, 0.4, 0.5, 0.6, 0.7, 0.8, 0.9)
SAT_EDGES = (0.1, 0.2, 0.3, 0.4, 0.5, 0.6, 0.7, 0.8, 0.9)
NSAT = len(SAT_EDGES)
NK = NSAT + 2                 # PE psum rows: 9 sat + val-0.5 + val-0.4


def _register_custom_ops():
    """Author + register fused DVE ops in the dve_ops registry at runtime
    (the repo list is read-only; registration is by-name so appending to the
    module-level OPS list is sufficient for table-gen and tracing)."""
    from concourse import dve_ops
    from concourse.dve_spec import (
        C0, C1, C2, Spec, Src0, Src1, Zero, One, _has_src1, lower, maxx,
        minn, select,
    )
    from concourse.dve_uop import DveOpSpec

    if hasattr(dve_ops, "HPA"):
        return dve_ops

    from operator import add as _add

    def _accref(body_fn):
        def ref(in0, in1, c0, c1, c2):
            b = body_fn(
                np.asarray(in0, np.float32),
                np.asarray(in1, np.float32) if in1 is not None else None,
                c0, c1, c2,
            ).astype(np.float32)
            return b, b.reshape(b.shape[0], -1).sum(axis=-1, keepdims=True)
        return ref

    # gate: 1 where u>=0 else PACK (routes the count into the high field)
    gate = select(Src0 >= Zero, One, C1)

    defs = [
        # hue pair, A-form: t = (Src0 + C0*Src1 <= 0); accum t*(1|C1 by sign)
        (
            "HPA",
            Spec(
                body=((Src0 + C0 * Src1) <= Zero) * gate,
                accum=_add,
                accum_init=Zero,
                reference=_accref(
                    lambda u, v, c0, c1, c2: ((u + np.float32(c0) * v) <= 0)
                    * np.where(u >= 0, 1.0, c1)
                ),
            ),
        ),
        # hue pair, B-form: t = (Src1 + C0*Src0 <= 0)
        (
            "HPB",
            Spec(
                body=((Src1 + C0 * Src0) <= Zero) * gate,
                accum=_add,
                accum_init=Zero,
                reference=_accref(
                    lambda u, v, c0, c1, c2: ((v + np.float32(c0) * u) <= 0)
                    * np.where(u >= 0, 1.0, c1)
                ),
            ),
        ),
    ]
    for name, spec in defs:
        row = 1 + len(dve_ops.OPS)
        shas = {}
        for ver in ("v3", "v4"):
            uops = lower(spec, ver=ver)
            shas[ver] = DveOpSpec(
                name=name, opcode=row, uops=uops, rd1_en=_has_src1(spec)
            ).sha(ver)
        op = dve_ops.DveOp(name, spec, False, uops_sha=shas)
        dve_ops.OPS.append(op)
        dve_ops.CUSTOM_DVE_SPECS[name] = spec
        dve_ops._SUB_OPCODE_FOR_NAME[name] = row
        setattr(dve_ops, name, op)
    return dve_ops


def _build():
    dve_ops = _register_custom_ops()
    nc = bacc.Bacc(
        "TRN2", target_bir_lowering=False, debug=False, num_devices=NCORES
    )
    xr = nc.dram_tensor("x_real", [BPC * C * P, F], F32, kind="ExternalInput").ap()
    xf = nc.dram_tensor("x_fake", [BPC * C * P, F], F32, kind="ExternalInput").ap()
    out = nc.dram_tensor("out", [NITER * P, ACCW], F32, kind="ExternalOutput").ap()
    sat_out = nc.dram_tensor("sat", [2 * NK, F], F32, kind="ExternalOutput").ap()

    SIGN = mybir.ActivationFunctionType.Sign
    COPY = mybir.ActivationFunctionType.Copy
    LN = mybir.ActivationFunctionType.Ln

    with tile.TileContext(nc) as tc:
        with tc.tile_pool(name="main", bufs=2) as io_pool, tc.tile_pool(
            name="tmp", bufs=1
        ) as tmp_pool, tc.tile_pool(name="ps", bufs=1, space="PSUM") as ppool:
            # per-edge bias tiles for ScalarE Sign activations (bias = -edge)
            sbias = []
            for e in SVAL_EDGES + (0.0,):       # 0.0: E-count sign(u)
                bt = tmp_pool.tile([P, 1], F32, tag=f"sb{int(e*10)}",
                                   name=f"sb{int(e*10)}")
                nc.gpsimd.memset(bt[:], -e)
                sbias.append(bt)
            # one-hot stationaries for the PE colsum (col j = ones)
            stats = []
            for j in range(NK):
                st = tmp_pool.tile([P, NK], F16, tag=f"st{j}", name=f"st{j}")
                nc.vector.memset(st[:], 0.0)
                nc.vector.memset(st[:, j : j + 1], 1.0)
                stats.append(st)
            psum = ppool.tile([P, F], F32, tag="psum")
            scr2b = tmp_pool.tile([P, 2, F], F16, tag="scr2b", name="scr2b")
            ind_rot = [
                tmp_pool.tile([P, 2, F], F16, tag=f"ind{k}", name=f"ind{k}")
                for k in range(2)
            ]
            # f32 scratch shared by the custom-DVE dummy outs and the final
            # PSUM readback
            scr = tmp_pool.tile([P, F], F32, tag="scr", name="scr")

            V = nc.vector
            S = nc.scalar
            for it in range(NITER):
                src = xr if it < BPC else xf
                bi = it % BPC

                def plane(c):
                    q = bi * C + c
                    return src[q * P : (q + 1) * P, :]

                r = io_pool.tile([P, F], F32, tag="r")
                g = io_pool.tile([P, F], F32, tag="g")
                bl = io_pool.tile([P, F], F32, tag="bl")
                nc.sync.dma_start(g[:], plane(1))
                nc.sync.dma_start(bl[:], plane(2))
                nc.sync.dma_start(r[:], plane(0))

                # u and mx16 live in [P, 2, F] pair tiles so ScalarE can run
                # one batched activation over two iterations' data
                if it % 2 == 0:
                    upair = io_pool.tile([P, 2, F], F32, tag="upair")
                    mxpair = io_pool.tile([P, 2, F], F16, tag="mxpair")
                    wpair = io_pool.tile([P, 2, F], F16, tag="wpair")
                u = upair[:, it % 2, :]
                mx16 = mxpair[:, it % 2, :]
                v = tmp_pool.tile([P, F], F32, tag="v", name="v")
                r16 = io_pool.tile([P, F], F16, tag="r16")
                g16 = io_pool.tile([P, F], F16, tag="g16")
                b16 = io_pool.tile([P, F], F16, tag="b16")
                m1 = tmp_pool.tile([P, F], F16, tag="m1", name="m1")
                mn1 = m1
                mn16 = io_pool.tile([P, F], F16, tag="mn16")
                rmx = io_pool.tile([P, F], F16, tag="rmx")
                ratio = io_pool.tile([P, F], F16, tag="ratio")
                acc = io_pool.tile([P, ACCW], F32, tag="acc")

                # ScalarE: fp16 casts of the three planes
                S.activation(g16[:], g[:], COPY)
                S.activation(b16[:], bl[:], COPY)
                S.activation(r16[:], r[:], COPY)

                # DVE: u, v in f32 (custom-DVE hue ops need f32 operands)
                V.tensor_tensor(u, g[:], bl[:], AF.subtract)
                V.tensor_tensor(v[:], bl[:], r[:], AF.subtract)

                def emit_minmax():
                    # fp16 min/max chains (2x DVE mode): ScalarE's Ln and
                    # Sign work hangs off mx16/mn16
                    V.tensor_tensor(m1[:], r16[:], g16[:], AF.max)
                    V.tensor_tensor(mx16, m1[:], b16[:], AF.max)
                    V.tensor_tensor(mn1[:], r16[:], g16[:], AF.min)
                    V.tensor_tensor(mn16[:], mn1[:], b16[:], AF.min)

                def emit_hue():
                    # hue pairs on (u, v): acc0..3
                    V._custom_dve(dve_ops.HPA, out=scr[:], in0=u, in1=v[:],
                                  s0=0.6, s1=PACK, accum_out=acc[:, 0:1])
                    V._custom_dve(dve_ops.HPB, out=scr[:], in0=u, in1=v[:],
                                  s0=0.8, s1=PACK, accum_out=acc[:, 1:2])
                    V._custom_dve(dve_ops.HPB, out=scr[:], in0=u, in1=v[:],
                                  s0=0.2, s1=PACK, accum_out=acc[:, 2:3])
                    V._custom_dve(dve_ops.HPB, out=scr[:], in0=u, in1=v[:],
                                  s0=-C23, s1=PACK, accum_out=acc[:, 3:4])

                if it == 0:
                    # iteration 0: the casts are still in flight on ScalarE;
                    # fill the DVE with the cast-independent hue block first
                    emit_hue()
                    emit_minmax()
                else:
                    emit_minmax()
                    emit_hue()
                # sat < c  <=>  mn/mx > 1-c  <=>  ln(mn) - ln(mx) > ln(1-c).
                # (ScalarE Reciprocal is blocked for accuracy; Ln shares the
                # natural_log act table with Sign and Copy -> no reloads.
                # ln(0) = -inf keeps the mn==0 pixels in the right bin.)
                S.activation(rmx[:], mx16, LN)
                S.activation(ratio[:], mn16[:], LN)
                w = wpair[:, it % 2, :]
                V.tensor_tensor(w, ratio[:], rmx[:], AF.subtract)
                # sat indicators (4x DVE mode, no accum) + PE colsum into
                # PSUM, batched over the iteration pair
                base = 0 if it < BPC else 32
                rows = slice(base, base + NK)
                if it % 2 == 1:
                    for j, c in enumerate(SAT_EDGES):
                        indt = ind_rot[j % 2]
                        V.tensor_scalar(indt[:], wpair[:],
                                        float(np.log(1.0 - c)), None,
                                        AF.is_gt)
                        for sl in range(2):
                            for cj in range(4):
                                nc.tensor.matmul(
                                    psum[rows, cj * 512 : (cj + 1) * 512],
                                    stats[j][:, :],
                                    indt[:, sl, cj * 512 : (cj + 1) * 512],
                                    start=False,
                                    stop=False,
                                )

                # val-0.5 and val-0.4 ride the PE path too (ScalarE -> PE).
                # The half's first val matmuls carry start=True, zeroing all
                # NK psum rows of the region (one-hot stationary writes 0s
                # to the other rows).
                for vj, ve in ((0, 0.5), (1, 0.4)):
                    indt = ind_rot[(NSAT + vj) % 2]
                    V.tensor_scalar(indt[:, 0, :], mx16, ve, None, AF.is_lt)
                    for cj in range(4):
                        nc.tensor.matmul(
                            psum[rows, cj * 512 : (cj + 1) * 512],
                            stats[NSAT + vj][:, :],
                            indt[:, 0, cj * 512 : (cj + 1) * 512],
                            start=(it % BPC == 0 and vj == 0),
                            stop=(it % BPC == BPC - 1 and vj == 1),
                        )

                if it % 2 == 1:
                    # ScalarE, batched over the iteration pair: val-9 + E
                    # acc10..18 (val sign-sums over 2*F on fp16 mx), acc4 (E)
                    for k in range(9):
                        if k in (3, 4):  # 0.4/0.5 are counted on the PE path
                            continue
                        S.activation(
                            scr2b[:], mxpair[:], SIGN, bias=sbias[k][:],
                            accum_out=acc[:, 10 + k : 11 + k],
                        )
                    S.activation(
                        scr2b[:], upair[:], SIGN,
                        bias=sbias[9][:], accum_out=acc[:, 4:5],
                    )
                nc.sync.dma_start(out[it * P : (it + 1) * P, :], acc[:, :])

            # final: stage the PE accumulator rows PSUM -> SBUF on the
            # (idle-by-now) ScalarE, then DMA out; the host sums the
            # 2048-wide rows (exact integer f32 sums)
            S.activation(scr[0:NK, :], psum[0:NK, :], COPY)
            S.activation(scr[32 : 32 + NK, :], psum[32 : 32 + NK, :], COPY)
            nc.sync.dma_start(sat_out[0:NK, :], scr[0:NK, :])
            nc.sync.dma_start(sat_out[NK : 2 * NK, :], scr[32 : 32 + NK, :])

    nc.compile()
    return nc


def _register_ntff_hook():
    """Register the axon NTFF profiling hook (the container's antenv stub
    lacks axon_hooks, so trn_boot's registration was skipped). Also keep
    profile artifacts local instead of uploading to a share."""
    import types

    import antenv

    if "antenv.axon_hooks" not in sys.modules:
        mod = types.ModuleType("antenv.axon_hooks")
        holder = [None]
        mod.set_axon_ntff_profile_hook = lambda h: holder.__setitem__(0, h)
        mod.get_axon_ntff_profile_hook = lambda: holder[0]
        sys.modules["antenv.axon_hooks"] = mod
        antenv.axon_hooks = mod
    from antenv import axon_hooks

    if axon_hooks.get_axon_ntff_profile_hook() is None:
        from trn_agent_boot.trn_boot import _ntff_profile_via_ctypes

        axon_hooks.set_axon_ntff_profile_hook(
            _ntff_profile_via_ctypes("/opt/axon/libaxon_pjrt.so")
        )
    bass_utils.upload_artifacts = lambda tmpdir: tmpdir


def _get_nc():
    if "nc" not in _CACHE:
        _CACHE["nc"] = _build()
    return _CACHE["nc"]


def kernel(x_real: np.ndarray, x_fake: np.ndarray) -> np.ndarray:
    global LAST_EXEC_NS
    nc = _get_nc()

    in_maps = []
    for c in range(NCORES):
        sl = slice(c * BPC, (c + 1) * BPC)
        in_maps.append(
            {
                "x_real": np.ascontiguousarray(x_real[sl]).reshape(BPC * C * P, F),
                "x_fake": np.ascontiguousarray(x_fake[sl]).reshape(BPC * C * P, F),
            }
        )

    import os

    trace = bool(int(os.environ.get("KERNEL_TRACE", "0")))
    if trace:
        _register_ntff_hook()
    res = bass_utils.run_bass_kernel_spmd(
        nc, in_maps, core_ids=list(range(NCORES)), trace=trace
    )
    LAST_EXEC_NS = res.exec_time_ns
    _CACHE["last_res"] = res

    # Decode.  Packed hue cols are exact ints in f32: split via % and //.
    # Sign-sum cols decode as N_lt = (N - S)/2.  Sat counts come from the
    # PE PSUM accumulators (already plain cumulative counts).
    PK = int(PACK)
    hue_lo = np.zeros((2, 4))      # A,B,C,D
    hue_hi = np.zeros((2, 4))      # F,G,H,I tilde counts (u<0 side)
    E_sign = np.zeros(2)
    sign_sums = np.zeros((2, 9))
    C_sat = np.zeros((2, 9))
    C_val05 = np.zeros(2)
    C_val04 = np.zeros(2)
    for core_out in res.results:
        o = np.asarray(core_out["out"]).reshape(NITER, P, ACCW).astype(np.int64)
        s = np.asarray(core_out["sat"]).sum(axis=1).astype(np.int64)
        C_sat[0] += s[0:NSAT]
        C_sat[1] += s[NK : NK + NSAT]
        C_val05[0] += s[NSAT]
        C_val05[1] += s[NK + NSAT]
        C_val04[0] += s[NSAT + 1]
        C_val04[1] += s[NK + NSAT + 1]
        for t, sl in ((0, slice(0, BPC)), (1, slice(BPC, NITER))):
            blk = o[sl]
            packed = blk[:, :, 0:4]
            hue_lo[t] += (packed % PK).sum(axis=(0, 1))
            hue_hi[t] += (packed // PK).sum(axis=(0, 1))
            # sign-sums live in the odd iterations of each half
            sign_sums[t] += blk[(1, 3), :, 10:19].sum(axis=(0, 1))
            E_sign[t] += blk[(1, 3), :, 4].sum() + blk[(1, 3), :, 5].sum()

    # hue cumulative counts [2, 9]
    E = (NPIX + E_sign) / 2.0
    C_hue = np.stack([
        hue_lo[:, 0], hue_lo[:, 1], hue_lo[:, 2], hue_lo[:, 3], E,
        NPIX - hue_hi[:, 0], NPIX - hue_hi[:, 1],
        NPIX - hue_hi[:, 2], NPIX - hue_hi[:, 3],
    ], axis=1)
    C_val = (NPIX - sign_sums) / 2.0
    C_val[:, 4] = C_val05
    C_val[:, 3] = C_val04

    loss = 0.0
    for wgt, Cc in ((ALPHA, C_hue), (BETA, C_sat), (GAMMA, C_val)):
        hist = np.zeros((2, 10))
        hist[:, 0] = Cc[:, 0]
        hist[:, 1:9] = Cc[:, 1:] - Cc[:, :-1]
        hist[:, 9] = NPIX - Cc[:, 8]
        loss += wgt * np.abs(hist[0] - hist[1]).mean()
    return np.asarray(loss, dtype=np.float32)
